# revision 36
# baseline (speedup 1.0000x reference)
"""Trainium2 Bass kernel for the MemoryEfficientMambaBlock problem.

Data-parallel over 8 NeuronCores: x sharded over tokens, small weights
replicated. Per core, per 512-token tile:
  LayerNorm (bn_stats + magic-rsqrt + apply, all on DVE, so the scalar
  engine only ever runs SiLU — no activation-table swaps) -> PE
  transpose to feature-major bf16 -> ACT copyback casting to fp8e4
  (gamma folded into W_proj host-side, beta folded into b_proj) ->
  fp8 DoubleRow matmul x8 @ (W_hi + W_lo) where W_hi/W_lo are a
  two-level e4m3 decomposition of W_proj*64 (kills the weight-side
  quantization noise; 8 pair-MMs per 128-col chunk accumulate in one
  PSUM bank) -> SiLU per m-chunk with the 1/(S_W*S_X) descale fused ->
  bf16 matmul @W_stateT -> SiLU+(b_state+initial_state) -> K=9 f32r
  matmul (ones row carries b_out) with the residual add fused into the
  DVE copyback.
"""

import sys

if "/opt/trn_rl_repo" not in sys.path:
    sys.path.insert(0, "/opt/trn_rl_repo")

import numpy as np
import ml_dtypes

import concourse.bass as bass
import concourse.mybir as mybir
import concourse.tile as tile
from concourse.bass_utils import run_bass_kernel_spmd

P = 128
D_MODEL = 1024
D_INNER = 2048
D_STATE = 8
EPS = 1e-5
N_CORES = 8
TOK_TOTAL = 2 * 128 * 196  # 50176
TOK = TOK_TOTAL // N_CORES  # 6272
TILE_T = 512
G = TILE_T // P  # 4

KD = D_MODEL // P  # 8 contraction chunks for matmul 1
ME = D_INNER // P  # 16 output tiles for matmul 1 / contraction chunks for 2

S_W = 64.0  # fp8 scale on W_proj
S_X = 8.0  # fp8 scale on xn (folded into rstd)

F32 = mybir.dt.float32
F32R = mybir.dt.float32r
BF16 = mybir.dt.bfloat16
FP8 = mybir.dt.float8e4
I32 = mybir.dt.int32

MAGIC = 0x5F3759DF


def _split_multi_waits(nc):
    """This container's walrus accepts at most ONE semaphore wait per
    instruction. Hoist all but the last wait of each instruction onto
    fresh single-wait NoOps inserted immediately before it on the same
    engine (the sequencer processes instructions in order, so semantics
    are unchanged)."""
    n_split = 0
    for f in nc.m.functions:
        for blk in f.blocks:
            out = []
            changed = False
            for inst in blk.instructions:
                si = inst.sync_info
                waits = list(si.on_wait) if si is not None else []
                if len(waits) > 1:
                    changed = True
                    for j, w in enumerate(waits[:-1]):
                        nop = mybir.InstNoOp(
                            name=f"{inst.name}-wsplit{j}", ins=[], outs=[]
                        )
                        nop.engine = inst.engine
                        nop.sync_info = mybir.SyncInfo(on_wait=[w], on_update=[])
                        out.append(nop)
                        n_split += 1
                    inst.sync_info = mybir.SyncInfo(
                        on_wait=[waits[-1]], on_update=list(si.on_update)
                    )
                out.append(inst)
            if changed:
                blk.instructions = out
    return n_split


def build_kernel(has_bias: bool):
    nc = bass.Bass()
    x = nc.dram_tensor("x", [TOK, D_MODEL], F32, kind="ExternalInput")
    # hi/lo two-level fp8 decomposition of W_proj^T * S_W
    wpt8 = nc.dram_tensor("wpt8", [2, D_MODEL, D_INNER], FP8, kind="ExternalInput")
    wst = nc.dram_tensor("wst", [D_INNER, D_STATE], BF16, kind="ExternalInput")
    wo9 = nc.dram_tensor("wo9", [D_STATE + 1, D_MODEL], F32R, kind="ExternalInput")
    bp = nc.dram_tensor("bp", [1, D_INNER], F32R, kind="ExternalInput")
    b2 = nc.dram_tensor("b2", [D_STATE, 1], F32, kind="ExternalInput")
    ones = nc.dram_tensor("ones", [1, TILE_T], F32R, kind="ExternalInput")
    ident_d = nc.dram_tensor("ident", [P, P], BF16, kind="ExternalInput")
    y = nc.dram_tensor("y", [TOK, D_MODEL], F32, kind="ExternalOutput")

    # all tiles full-size; the last tile overlaps the previous one so the
    # matmuls always stream N=512
    tiles = [(o, TILE_T) for o in range(0, TOK - TILE_T + 1, TILE_T)]
    if tiles[-1][0] + TILE_T < TOK:
        tiles.append((TOK - TILE_T, TILE_T))

    with tile.TileContext(nc) as tc:
        with (
            tc.tile_pool(name="singles", bufs=1) as singles,
            tc.tile_pool(name="xpool", bufs=3) as xpool,
            tc.tile_pool(name="xnpool", bufs=2) as xnpool,
            tc.tile_pool(name="xtpool", bufs=2) as xtpool,
            tc.tile_pool(name="projp", bufs=2) as projp,
            tc.tile_pool(name="outp", bufs=2) as outp,
            tc.tile_pool(name="statp", bufs=6) as statp,
            tc.tile_pool(name="ps_tr", bufs=2, space="PSUM") as ps_tr,
            tc.tile_pool(name="ps1", bufs=3, space="PSUM") as ps1,
            tc.tile_pool(name="ps2", bufs=1, space="PSUM") as ps2,
            tc.tile_pool(name="ps3", bufs=1, space="PSUM") as ps3,
        ):
            def a_dma(i):
                off, T = tiles[i]
                x_sb = xpool.tile([P, G, D_MODEL], F32, tag="x")
                q = nc.sync if i % 2 == 0 else nc.scalar
                q.dma_start(
                    x_sb, x[off : off + T, :].rearrange("(g p) d -> p g d", p=P)
                )
                return x_sb

            def a_ln(x_sb):
                """layernorm one tile -> xn16 (token-major bf16, scaled by
                S_X via rstd = rsqrt((var+eps)/S_X^2)); all on DVE"""
                stats = statp.tile([P, G, 2, 6], F32, tag="bnst")
                mv = statp.tile([P, G, 2], F32, tag="mv")
                for g in range(G):
                    nc.vector.bn_stats(stats[:, g, 0, :], x_sb[:, g, 0:512])
                    nc.vector.bn_stats(stats[:, g, 1, :], x_sb[:, g, 512:1024])
                    nc.vector.bn_aggr(mv[:, g, :], stats[:, g])
                vp = statp.tile([P, G], F32, tag="vp")
                nc.vector.tensor_scalar(
                    out=vp,
                    in0=mv[:, :, 1],
                    scalar1=EPS,
                    scalar2=1.0 / (S_X * S_X),
                    op0=mybir.AluOpType.add,
                    op1=mybir.AluOpType.mult,
                )
                # magic rsqrt + two Newton steps (rel err ~5e-6)
                rs = statp.tile([P, G], F32, tag="rs")
                nc.vector.tensor_scalar(
                    out=rs.bitcast(I32),
                    in0=vp.bitcast(I32),
                    scalar1=1,
                    scalar2=None,
                    op0=mybir.AluOpType.arith_shift_right,
                )
                nc.vector.tensor_scalar(
                    out=rs.bitcast(I32),
                    in0=rs.bitcast(I32),
                    scalar1=-1,
                    scalar2=MAGIC,
                    op0=mybir.AluOpType.mult,
                    op1=mybir.AluOpType.add,
                )
                sq = statp.tile([P, G], F32, tag="sq")
                for _ in range(2):
                    nc.vector.tensor_tensor(
                        out=sq, in0=rs, in1=rs, op=mybir.AluOpType.mult
                    )
                    nc.vector.tensor_tensor(
                        out=sq, in0=sq, in1=vp, op=mybir.AluOpType.mult
                    )
                    nc.vector.tensor_scalar(
                        out=sq,
                        in0=sq,
                        scalar1=-0.5,
                        scalar2=1.5,
                        op0=mybir.AluOpType.mult,
                        op1=mybir.AluOpType.add,
                    )
                    nc.vector.tensor_tensor(
                        out=rs, in0=rs, in1=sq, op=mybir.AluOpType.mult
                    )
                xn_sb = xnpool.tile([P, G, D_MODEL], BF16, tag="xn")
                for g in range(G):
                    nc.vector.tensor_scalar(
                        out=xn_sb[:, g, :],
                        in0=x_sb[:, g, :],
                        scalar1=mv[:, g, 0:1],
                        scalar2=rs[:, g : g + 1],
                        op0=mybir.AluOpType.subtract,
                        op1=mybir.AluOpType.mult,
                    )
                return xn_sb

            def a_tr(xn_sb):
                """PE-transpose to feature-major; ACT copyback casts to
                fp8e4. One PSUM tile and one ACT copy per k-pair."""
                xnT = xtpool.tile([P, KD, G, P], FP8, tag="xnT")
                for r in range(KD // 2):
                    ptr = ps_tr.tile([P, 2, G, P], BF16, tag="ptr")
                    for kk in range(2):
                        k = 2 * r + kk
                        for g in range(G):
                            nc.tensor.transpose(
                                ptr[:, kk, g, :],
                                xn_sb[:, g, k * P : (k + 1) * P],
                                ident,
                            )
                    nc.scalar.activation(
                        out=xnT[:, 2 * r : 2 * r + 2],
                        in_=ptr[:],
                        func=mybir.ActivationFunctionType.Copy,
                    )
                return xnT

            # software pipeline: x-DMA two tiles ahead, LayerNorm one tile
            # ahead, transposes one tile ahead in the M2->M3 pocket.
            # x0/x1 lead their queues so tile 0's LN/transpose chain
            # starts immediately; the weight chunks fill in behind and
            # arrive before matmul 1 consumes them.
            ident = singles.tile([P, P], BF16)
            nc.scalar.dma_start(ident, ident_d[:, :])
            x_tiles = [a_dma(0), a_dma(1)]
            xn_cur = a_ln(x_tiles[0])
            xnT_cur = a_tr(xn_cur)
            wpt_sb = singles.tile([P, 2, KD, D_INNER], FP8)
            wpt_r = wpt8[:, :, :].rearrange("h (k p) e -> p h k e", p=P)
            qs = [nc.sync, nc.scalar]
            for h in range(2):
                for k in range(KD):
                    qs[k % 2].dma_start(wpt_sb[:, h, k], wpt_r[:, h, k])
            wst_sb = singles.tile([P, ME, D_STATE], BF16)
            nc.sync.dma_start(wst_sb, wst[:, :].rearrange("(k p) s -> p k s", p=P))
            wo9_sb = singles.tile([D_STATE + 1, D_MODEL], F32R)
            nc.scalar.dma_start(wo9_sb, wo9[:, :])
            b2_sb = singles.tile([D_STATE, 1], F32)
            nc.sync.dma_start(b2_sb, b2[:, :])
            if has_bias:
                bp_sb = singles.tile([1, D_INNER], F32R)
                nc.sync.dma_start(bp_sb, bp[:, :])
                ones_sb = singles.tile([1, TILE_T], F32R)
                nc.scalar.dma_start(ones_sb, ones[:, :])
            xn_next = a_ln(x_tiles[1])
            for i, (off, T) in enumerate(tiles):
                x_sb = x_tiles[i]
                xnT = xnT_cur
                if i + 2 < len(tiles):
                    x_tiles.append(a_dma(i + 2))
                # cs9 allocated + ones row DMA'd early (row 8 is only
                # reachable by DMA; issuing here hides its latency)
                cs9 = statp.tile([D_STATE + 1, TILE_T], F32R, tag="cs9")
                nc.sync.dma_start(cs9[D_STATE : D_STATE + 1, :], ones[:, :])
                # matmul 1: fp8 DoubleRow over (hi, lo) weight levels.
                # Two m-chunks interleave across two PSUM banks so the
                # accumulation-group start/stop bubble of one chunk hides
                # under the other's steady MMs. SiLU per m-chunk with the
                # 1/(S_W*S_X) descale fused.
                projT = projp.tile([P, ME, TILE_T], BF16, tag="projT")
                for q in range(ME // 2):
                    pair = (2 * q, 2 * q + 1)
                    p1_even = ps1.tile([P, TILE_T], F32, tag="p1")
                    p1_odd = ps1.tile([P, TILE_T], F32, tag="p1")
                    p1s = [p1_even, p1_odd]
                    if has_bias:
                        for j, m in enumerate(pair):
                            nc.tensor.matmul(
                                p1s[j],
                                lhsT=bp_sb[:, m * P : (m + 1) * P],
                                rhs=ones_sb[:, :],
                                start=True,
                                stop=False,
                                skip_group_check=True,
                            )
                    for h in range(2):
                        for r in range(KD // 2):
                            for j, m in enumerate(pair):
                                nc.tensor.matmul(
                                    p1s[j],
                                    lhsT=wpt_sb[
                                        :, h, 2 * r : 2 * r + 2, m * P : (m + 1) * P
                                    ],
                                    rhs=xnT[:, 2 * r : 2 * r + 2],
                                    start=(h == 0 and r == 0 and not has_bias),
                                    stop=(h == 1 and r == KD // 2 - 1),
                                    perf_mode=mybir.MatmulPerfMode.DoubleRow,
                                    skip_group_check=True,
                                )
                    for j, m in enumerate(pair):
                        nc.scalar.activation(
                            out=projT[:, m, :],
                            in_=p1s[j],
                            func=mybir.ActivationFunctionType.Silu,
                            bias=0.0,
                            scale=1.0 / (S_W * S_X),
                        )
                # matmul 2: bf16, [D_STATE, T]
                p2 = ps2.tile([D_STATE, TILE_T], F32, tag="p2")
                for k2 in range(ME):
                    nc.tensor.matmul(
                        p2,
                        lhsT=wst_sb[:, k2, :],
                        rhs=projT[:, k2, :],
                        start=(k2 == 0),
                        stop=(k2 == ME - 1),
                    )
                # next tile's transposes fill the PE while ACT drains
                # p2 -> cs9; LN for the tile after runs on DVE
                if i + 1 < len(tiles):
                    xnT_cur = a_tr(xn_next)
                if i + 2 < len(tiles):
                    xn_next = a_ln(x_tiles[i + 2])
                nc.scalar.activation(
                    out=cs9[:D_STATE, :],
                    in_=p2,
                    func=mybir.ActivationFunctionType.Silu,
                    bias=b2_sb,
                    scale=1.0,
                )
                # matmul 3: K=9 (ones row adds b_out), token-major out;
                # residual add fused into the DVE copyback
                out_sb = outp.tile([P, G, D_MODEL], F32, tag="out")
                for g in range(G):
                    p3 = ps3.tile([P, 2, 512], F32, tag="p3")
                    for h in range(D_MODEL // 512):
                        nc.tensor.matmul(
                            p3[:, h, :],
                            lhsT=cs9[:, g * P : (g + 1) * P],
                            rhs=wo9_sb[:, h * 512 : (h + 1) * 512],
                            start=True,
                            stop=True,
                        )
                        nc.vector.tensor_add(
                            out=out_sb[:, g, h * 512 : (h + 1) * 512],
                            in0=p3[:, h, :],
                            in1=x_sb[:, g, h * 512 : (h + 1) * 512],
                        )
                nc.sync.dma_start(
                    y[off : off + T, :].rearrange("(g p) d -> p g d", p=P), out_sb
                )

    _split_multi_waits(nc)
    return nc


_NC_CACHE = {}


def _get_nc(has_bias: bool):
    if has_bias not in _NC_CACHE:
        _NC_CACHE[has_bias] = build_kernel(has_bias)
    return _NC_CACHE[has_bias]


def make_in_maps(inputs):
    x = np.ascontiguousarray(inputs["x"], dtype=np.float32).reshape(-1, D_MODEL)
    W_proj = np.asarray(inputs["W_proj"], dtype=np.float64)
    b_proj = np.asarray(inputs["b_proj"], dtype=np.float64)
    W_state = np.asarray(inputs["W_state"], dtype=np.float32)
    b_state = np.asarray(inputs["b_state"], dtype=np.float32)
    W_out = np.asarray(inputs["W_out"], dtype=np.float32)
    b_out = np.asarray(inputs["b_out"], dtype=np.float32)
    initial_state = np.asarray(inputs["initial_state"], dtype=np.float32)
    gamma = np.asarray(inputs["gamma"], dtype=np.float64)
    beta = np.asarray(inputs["beta"], dtype=np.float64)

    # fold the LayerNorm affine into the projection
    Wp = W_proj * gamma[None, :]  # [d_inner, d_model]
    bp = b_proj + W_proj @ beta  # [d_inner]
    has_bias = bool(np.any(bp != 0.0))

    wpre = np.clip(Wp.T * S_W, -224.0, 224.0)
    w_hi = wpre.astype(ml_dtypes.float8_e4m3)
    w_lo = (wpre - w_hi.astype(np.float64)).astype(ml_dtypes.float8_e4m3)
    wpt8 = np.stack([w_hi, w_lo], axis=0)  # [2, d_model, d_inner]

    shared = {
        "wpt8": np.ascontiguousarray(wpt8),
        "wst": np.ascontiguousarray(W_state.T.astype(ml_dtypes.bfloat16)),
        "wo9": np.ascontiguousarray(
            np.concatenate([W_out.T, b_out[None, :]], axis=0)
        ),
        "bp": np.ascontiguousarray((bp * S_W * S_X).astype(np.float32))[None, :],
        "b2": np.ascontiguousarray(
            (b_state + initial_state.reshape(-1)).reshape(D_STATE, 1)
        ),
        "ones": np.ones((1, TILE_T), dtype=np.float32),
        "ident": np.eye(P, dtype=ml_dtypes.bfloat16),
    }
    in_maps = []
    for c in range(N_CORES):
        m = {"x": np.ascontiguousarray(x[c * TOK : (c + 1) * TOK])}
        m.update(shared)
        in_maps.append(m)
    return in_maps, has_bias


def kernel(**inputs) -> np.ndarray:
    in_maps, has_bias = make_in_maps(inputs)
    nc = _get_nc(has_bias)
    res = run_bass_kernel_spmd(nc, in_maps, core_ids=list(range(N_CORES)))
    out = np.concatenate([res.results[c]["y"] for c in range(N_CORES)], axis=0)
    return out.reshape(np.asarray(inputs["x"]).shape)


# revision 39
# speedup vs baseline: 1.0069x; 1.0069x over previous
"""Trainium2 Bass kernel for the MemoryEfficientMambaBlock problem.

Data-parallel over 8 NeuronCores: x sharded over tokens, small weights
replicated. Per core, per 512-token tile:
  LayerNorm (bn_stats + magic-rsqrt + apply, all on DVE, so the scalar
  engine only ever runs SiLU — no activation-table swaps) -> PE
  transpose to feature-major bf16 -> ACT copyback casting to fp8e4
  (gamma folded into W_proj host-side, beta folded into b_proj) ->
  fp8 DoubleRow matmul x8 @ (W_hi + W_lo) where W_hi/W_lo are a
  two-level e4m3 decomposition of W_proj*64 (kills the weight-side
  quantization noise; 8 pair-MMs per 128-col chunk accumulate in one
  PSUM bank) -> SiLU per m-chunk with the 1/(S_W*S_X) descale fused ->
  bf16 matmul @W_stateT -> SiLU+(b_state+initial_state) -> K=9 f32r
  matmul (ones row carries b_out) with the residual add fused into the
  DVE copyback.
"""

import sys

if "/opt/trn_rl_repo" not in sys.path:
    sys.path.insert(0, "/opt/trn_rl_repo")

import numpy as np
import ml_dtypes

import concourse.bass as bass
import concourse.mybir as mybir
import concourse.tile as tile
from concourse.bass_utils import run_bass_kernel_spmd

P = 128
D_MODEL = 1024
D_INNER = 2048
D_STATE = 8
EPS = 1e-5
N_CORES = 8
TOK_TOTAL = 2 * 128 * 196  # 50176
TOK = TOK_TOTAL // N_CORES  # 6272
TILE_T = 512
G = TILE_T // P  # 4

KD = D_MODEL // P  # 8 contraction chunks for matmul 1
ME = D_INNER // P  # 16 output tiles for matmul 1 / contraction chunks for 2

S_W = 64.0  # fp8 scale on W_proj
S_X = 8.0  # fp8 scale on xn (folded into rstd)

F32 = mybir.dt.float32
F32R = mybir.dt.float32r
BF16 = mybir.dt.bfloat16
FP8 = mybir.dt.float8e4
I32 = mybir.dt.int32

MAGIC = 0x5F3759DF


def _split_multi_waits(nc):
    """This container's walrus accepts at most ONE semaphore wait per
    instruction. Hoist all but the last wait of each instruction onto
    fresh single-wait NoOps inserted immediately before it on the same
    engine (the sequencer processes instructions in order, so semantics
    are unchanged)."""
    n_split = 0
    for f in nc.m.functions:
        for blk in f.blocks:
            out = []
            changed = False
            for inst in blk.instructions:
                si = inst.sync_info
                waits = list(si.on_wait) if si is not None else []
                if len(waits) > 1:
                    changed = True
                    for j, w in enumerate(waits[:-1]):
                        nop = mybir.InstNoOp(
                            name=f"{inst.name}-wsplit{j}", ins=[], outs=[]
                        )
                        nop.engine = inst.engine
                        nop.sync_info = mybir.SyncInfo(on_wait=[w], on_update=[])
                        out.append(nop)
                        n_split += 1
                    inst.sync_info = mybir.SyncInfo(
                        on_wait=[waits[-1]], on_update=list(si.on_update)
                    )
                out.append(inst)
            if changed:
                blk.instructions = out
    return n_split


def build_kernel(has_bias: bool):
    nc = bass.Bass()
    x = nc.dram_tensor("x", [TOK, D_MODEL], F32, kind="ExternalInput")
    # hi/lo two-level fp8 decomposition of W_proj^T * S_W
    wpt8 = nc.dram_tensor("wpt8", [2, D_MODEL, D_INNER], FP8, kind="ExternalInput")
    wst = nc.dram_tensor("wst", [D_INNER, D_STATE], BF16, kind="ExternalInput")
    wo9 = nc.dram_tensor("wo9", [D_STATE + 1, D_MODEL], F32R, kind="ExternalInput")
    bp = nc.dram_tensor("bp", [1, D_INNER], F32R, kind="ExternalInput")
    b2 = nc.dram_tensor("b2", [D_STATE, 1], F32, kind="ExternalInput")
    ones = nc.dram_tensor("ones", [1, TILE_T], F32R, kind="ExternalInput")
    ident_d = nc.dram_tensor("ident", [P, P], BF16, kind="ExternalInput")
    y = nc.dram_tensor("y", [TOK, D_MODEL], F32, kind="ExternalOutput")

    # all tiles full-size; the last tile overlaps the previous one so the
    # matmuls always stream N=512
    tiles = [(o, TILE_T) for o in range(0, TOK - TILE_T + 1, TILE_T)]
    if tiles[-1][0] + TILE_T < TOK:
        tiles.append((TOK - TILE_T, TILE_T))

    with tile.TileContext(nc) as tc:
        with (
            tc.tile_pool(name="singles", bufs=1) as singles,
            tc.tile_pool(name="xpool", bufs=3) as xpool,
            tc.tile_pool(name="xnpool", bufs=2) as xnpool,
            tc.tile_pool(name="xtpool", bufs=2) as xtpool,
            tc.tile_pool(name="projp", bufs=2) as projp,
            tc.tile_pool(name="outp", bufs=2) as outp,
            tc.tile_pool(name="statp", bufs=6) as statp,
            tc.tile_pool(name="ps_tr", bufs=2, space="PSUM") as ps_tr,
            tc.tile_pool(name="ps1", bufs=3, space="PSUM") as ps1,
            tc.tile_pool(name="ps2", bufs=1, space="PSUM") as ps2,
            tc.tile_pool(name="ps3", bufs=1, space="PSUM") as ps3,
        ):
            def a_dma(i):
                off, T = tiles[i]
                x_sb = xpool.tile([P, G, D_MODEL], F32, tag="x")
                q = nc.sync if i % 2 == 0 else nc.scalar
                q.dma_start(
                    x_sb, x[off : off + T, :].rearrange("(g p) d -> p g d", p=P)
                )
                return x_sb

            def a_ln(x_sb):
                """layernorm one tile -> xn16 (token-major bf16, scaled by
                S_X via rstd = rsqrt((var+eps)/S_X^2)); all on DVE"""
                stats = statp.tile([P, G, 2, 6], F32, tag="bnst")
                mv = statp.tile([P, G, 2], F32, tag="mv")
                for g in range(G):
                    nc.vector.bn_stats(stats[:, g, 0, :], x_sb[:, g, 0:512])
                    nc.vector.bn_stats(stats[:, g, 1, :], x_sb[:, g, 512:1024])
                    nc.vector.bn_aggr(mv[:, g, :], stats[:, g])
                vp = statp.tile([P, G], F32, tag="vp")
                nc.vector.tensor_scalar(
                    out=vp,
                    in0=mv[:, :, 1],
                    scalar1=EPS,
                    scalar2=1.0 / (S_X * S_X),
                    op0=mybir.AluOpType.add,
                    op1=mybir.AluOpType.mult,
                )
                # magic rsqrt + two Newton steps (rel err ~5e-6)
                rs = statp.tile([P, G], F32, tag="rs")
                nc.vector.tensor_scalar(
                    out=rs.bitcast(I32),
                    in0=vp.bitcast(I32),
                    scalar1=1,
                    scalar2=None,
                    op0=mybir.AluOpType.arith_shift_right,
                )
                nc.vector.tensor_scalar(
                    out=rs.bitcast(I32),
                    in0=rs.bitcast(I32),
                    scalar1=-1,
                    scalar2=MAGIC,
                    op0=mybir.AluOpType.mult,
                    op1=mybir.AluOpType.add,
                )
                sq = statp.tile([P, G], F32, tag="sq")
                for _ in range(2):
                    nc.vector.tensor_tensor(
                        out=sq, in0=rs, in1=rs, op=mybir.AluOpType.mult
                    )
                    nc.vector.tensor_tensor(
                        out=sq, in0=sq, in1=vp, op=mybir.AluOpType.mult
                    )
                    nc.vector.tensor_scalar(
                        out=sq,
                        in0=sq,
                        scalar1=-0.5,
                        scalar2=1.5,
                        op0=mybir.AluOpType.mult,
                        op1=mybir.AluOpType.add,
                    )
                    nc.vector.tensor_tensor(
                        out=rs, in0=rs, in1=sq, op=mybir.AluOpType.mult
                    )
                xn_sb = xnpool.tile([P, G, D_MODEL], BF16, tag="xn")
                for g in range(G):
                    nc.vector.tensor_scalar(
                        out=xn_sb[:, g, :],
                        in0=x_sb[:, g, :],
                        scalar1=mv[:, g, 0:1],
                        scalar2=rs[:, g : g + 1],
                        op0=mybir.AluOpType.subtract,
                        op1=mybir.AluOpType.mult,
                    )
                return xn_sb

            def a_tr(xn_sb):
                """PE-transpose to feature-major; ACT copyback casts to
                fp8e4. One PSUM tile and one ACT copy per k-pair."""
                xnT = xtpool.tile([P, KD, G, P], FP8, tag="xnT")
                for r in range(KD // 2):
                    ptr = ps_tr.tile([P, 2, G, P], BF16, tag="ptr")
                    for kk in range(2):
                        k = 2 * r + kk
                        for g in range(G):
                            nc.tensor.transpose(
                                ptr[:, kk, g, :],
                                xn_sb[:, g, k * P : (k + 1) * P],
                                ident,
                            )
                    nc.scalar.activation(
                        out=xnT[:, 2 * r : 2 * r + 2],
                        in_=ptr[:],
                        func=mybir.ActivationFunctionType.Copy,
                    )
                return xnT

            # software pipeline: x-DMA two tiles ahead, LayerNorm one tile
            # ahead, transposes one tile ahead in the M2->M3 pocket.
            # x0/x1 lead their queues so tile 0's LN/transpose chain
            # starts immediately; the weight chunks fill in behind and
            # arrive before matmul 1 consumes them.
            ident = singles.tile([P, P], BF16)
            nc.scalar.dma_start(ident, ident_d[:, :])
            x_tiles = [a_dma(0), a_dma(1)]
            xn_cur = a_ln(x_tiles[0])
            xnT_cur = a_tr(xn_cur)
            wpt_sb = singles.tile([P, 2, KD, D_INNER], FP8)
            wpt_r = wpt8[:, :, :].rearrange("h (k p) e -> p h k e", p=P)
            qs = [nc.sync, nc.scalar]
            for h in range(2):
                for k in range(KD):
                    qs[k % 2].dma_start(wpt_sb[:, h, k], wpt_r[:, h, k])
            wst_sb = singles.tile([P, ME, D_STATE], BF16)
            nc.sync.dma_start(wst_sb, wst[:, :].rearrange("(k p) s -> p k s", p=P))
            wo9_sb = singles.tile([D_STATE + 1, D_MODEL], F32R)
            nc.scalar.dma_start(wo9_sb, wo9[:, :])
            b2_sb = singles.tile([D_STATE, 1], F32)
            nc.sync.dma_start(b2_sb, b2[:, :])
            if has_bias:
                bp_sb = singles.tile([1, D_INNER], F32R)
                nc.sync.dma_start(bp_sb, bp[:, :])
                ones_sb = singles.tile([1, TILE_T], F32R)
                nc.scalar.dma_start(ones_sb, ones[:, :])
            xn_next = a_ln(x_tiles[1])
            for i, (off, T) in enumerate(tiles):
                x_sb = x_tiles[i]
                xnT = xnT_cur
                if i + 2 < len(tiles):
                    x_tiles.append(a_dma(i + 2))
                # cs9 allocated + ones row DMA'd early (row 8 is only
                # reachable by DMA; issuing here hides its latency)
                cs9 = statp.tile([D_STATE + 1, TILE_T], F32R, tag="cs9")
                nc.sync.dma_start(cs9[D_STATE : D_STATE + 1, :], ones[:, :])
                # matmul 1: fp8 DoubleRow over (hi, lo) weight levels;
                # SiLU per m-chunk with the 1/(S_W*S_X) descale fused
                projT = projp.tile([P, ME, TILE_T], BF16, tag="projT")
                for m in range(ME):
                    p1 = ps1.tile([P, TILE_T], F32, tag="p1")
                    if has_bias:
                        nc.tensor.matmul(
                            p1,
                            lhsT=bp_sb[:, m * P : (m + 1) * P],
                            rhs=ones_sb[:, :],
                            start=True,
                            stop=False,
                            skip_group_check=True,
                        )
                    for h in range(2):
                        for r in range(KD // 2):
                            nc.tensor.matmul(
                                p1,
                                lhsT=wpt_sb[
                                    :, h, 2 * r : 2 * r + 2, m * P : (m + 1) * P
                                ],
                                rhs=xnT[:, 2 * r : 2 * r + 2],
                                start=(h == 0 and r == 0 and not has_bias),
                                stop=(h == 1 and r == KD // 2 - 1),
                                perf_mode=mybir.MatmulPerfMode.DoubleRow,
                                skip_group_check=has_bias,
                            )
                    nc.scalar.activation(
                        out=projT[:, m, :],
                        in_=p1,
                        func=mybir.ActivationFunctionType.Silu,
                        bias=0.0,
                        scale=1.0 / (S_W * S_X),
                    )
                # matmul 2: bf16, [D_STATE, T]
                p2 = ps2.tile([D_STATE, TILE_T], F32, tag="p2")
                for k2 in range(ME):
                    nc.tensor.matmul(
                        p2,
                        lhsT=wst_sb[:, k2, :],
                        rhs=projT[:, k2, :],
                        start=(k2 == 0),
                        stop=(k2 == ME - 1),
                    )
                # next tile's transposes fill the PE while ACT drains
                # p2 -> cs9; LN for the tile after runs on DVE
                if i + 1 < len(tiles):
                    xnT_cur = a_tr(xn_next)
                if i + 2 < len(tiles):
                    xn_next = a_ln(x_tiles[i + 2])
                nc.scalar.activation(
                    out=cs9[:D_STATE, :],
                    in_=p2,
                    func=mybir.ActivationFunctionType.Silu,
                    bias=b2_sb,
                    scale=1.0,
                )
                # matmul 3: K=9 (ones row adds b_out), token-major out;
                # residual add fused into the DVE copyback
                out_sb = outp.tile([P, G, D_MODEL], F32, tag="out")
                for g in range(G):
                    p3 = ps3.tile([P, 2, 512], F32, tag="p3")
                    for h in range(D_MODEL // 512):
                        nc.tensor.matmul(
                            p3[:, h, :],
                            lhsT=cs9[:, g * P : (g + 1) * P],
                            rhs=wo9_sb[:, h * 512 : (h + 1) * 512],
                            start=True,
                            stop=True,
                        )
                        nc.vector.tensor_add(
                            out=out_sb[:, g, h * 512 : (h + 1) * 512],
                            in0=p3[:, h, :],
                            in1=x_sb[:, g, h * 512 : (h + 1) * 512],
                        )
                nc.sync.dma_start(
                    y[off : off + T, :].rearrange("(g p) d -> p g d", p=P), out_sb
                )

    _split_multi_waits(nc)
    return nc


_NC_CACHE = {}


def _get_nc(has_bias: bool):
    if has_bias not in _NC_CACHE:
        _NC_CACHE[has_bias] = build_kernel(has_bias)
    return _NC_CACHE[has_bias]


def make_in_maps(inputs):
    x = np.ascontiguousarray(inputs["x"], dtype=np.float32).reshape(-1, D_MODEL)
    W_proj = np.asarray(inputs["W_proj"], dtype=np.float64)
    b_proj = np.asarray(inputs["b_proj"], dtype=np.float64)
    W_state = np.asarray(inputs["W_state"], dtype=np.float32)
    b_state = np.asarray(inputs["b_state"], dtype=np.float32)
    W_out = np.asarray(inputs["W_out"], dtype=np.float32)
    b_out = np.asarray(inputs["b_out"], dtype=np.float32)
    initial_state = np.asarray(inputs["initial_state"], dtype=np.float32)
    gamma = np.asarray(inputs["gamma"], dtype=np.float64)
    beta = np.asarray(inputs["beta"], dtype=np.float64)

    # fold the LayerNorm affine into the projection
    Wp = W_proj * gamma[None, :]  # [d_inner, d_model]
    bp = b_proj + W_proj @ beta  # [d_inner]
    has_bias = bool(np.any(bp != 0.0))

    wpre = np.clip(Wp.T * S_W, -224.0, 224.0)
    w_hi = wpre.astype(ml_dtypes.float8_e4m3)
    w_lo = (wpre - w_hi.astype(np.float64)).astype(ml_dtypes.float8_e4m3)
    wpt8 = np.stack([w_hi, w_lo], axis=0)  # [2, d_model, d_inner]

    shared = {
        "wpt8": np.ascontiguousarray(wpt8),
        "wst": np.ascontiguousarray(W_state.T.astype(ml_dtypes.bfloat16)),
        "wo9": np.ascontiguousarray(
            np.concatenate([W_out.T, b_out[None, :]], axis=0)
        ),
        "bp": np.ascontiguousarray((bp * S_W * S_X).astype(np.float32))[None, :],
        "b2": np.ascontiguousarray(
            (b_state + initial_state.reshape(-1)).reshape(D_STATE, 1)
        ),
        "ones": np.ones((1, TILE_T), dtype=np.float32),
        "ident": np.eye(P, dtype=ml_dtypes.bfloat16),
    }
    in_maps = []
    for c in range(N_CORES):
        m = {"x": np.ascontiguousarray(x[c * TOK : (c + 1) * TOK])}
        m.update(shared)
        in_maps.append(m)
    return in_maps, has_bias


def kernel(**inputs) -> np.ndarray:
    in_maps, has_bias = make_in_maps(inputs)
    nc = _get_nc(has_bias)
    res = run_bass_kernel_spmd(nc, in_maps, core_ids=list(range(N_CORES)))
    out = np.concatenate([res.results[c]["y"] for c in range(N_CORES)], axis=0)
    return out.reshape(np.asarray(inputs["x"]).shape)


# revision 40
# speedup vs baseline: 1.0076x; 1.0007x over previous
"""Trainium2 Bass kernel for the MemoryEfficientMambaBlock problem.

Data-parallel over 8 NeuronCores: x sharded over tokens, small weights
replicated. Per core, per 512-token tile:
  LayerNorm (bn_stats + magic-rsqrt + apply, all on DVE, so the scalar
  engine only ever runs SiLU — no activation-table swaps) -> PE
  transpose to feature-major bf16 -> ACT copyback casting to fp8e4
  (gamma folded into W_proj host-side, beta folded into b_proj) ->
  fp8 DoubleRow matmul x8 @ (W_hi + W_lo) where W_hi/W_lo are a
  two-level e4m3 decomposition of W_proj*64 (kills the weight-side
  quantization noise; 8 pair-MMs per 128-col chunk accumulate in one
  PSUM bank) -> SiLU per m-chunk with the 1/(S_W*S_X) descale fused ->
  bf16 matmul @W_stateT -> SiLU+(b_state+initial_state) -> K=9 f32r
  matmul (ones row carries b_out) with the residual add fused into the
  DVE copyback.
"""

import sys

if "/opt/trn_rl_repo" not in sys.path:
    sys.path.insert(0, "/opt/trn_rl_repo")

import numpy as np
import ml_dtypes

import concourse.bass as bass
import concourse.mybir as mybir
import concourse.tile as tile
from concourse.bass_utils import run_bass_kernel_spmd

P = 128
D_MODEL = 1024
D_INNER = 2048
D_STATE = 8
EPS = 1e-5
N_CORES = 8
TOK_TOTAL = 2 * 128 * 196  # 50176
TOK = TOK_TOTAL // N_CORES  # 6272
TILE_T = 512
G = TILE_T // P  # 4

KD = D_MODEL // P  # 8 contraction chunks for matmul 1
ME = D_INNER // P  # 16 output tiles for matmul 1 / contraction chunks for 2

S_W = 64.0  # fp8 scale on W_proj
S_X = 8.0  # fp8 scale on xn (folded into rstd)

F32 = mybir.dt.float32
F32R = mybir.dt.float32r
BF16 = mybir.dt.bfloat16
FP8 = mybir.dt.float8e4
I32 = mybir.dt.int32

MAGIC = 0x5F3759DF


def _split_multi_waits(nc):
    """This container's walrus accepts at most ONE semaphore wait per
    instruction. Hoist all but the last wait of each instruction onto
    fresh single-wait NoOps inserted immediately before it on the same
    engine (the sequencer processes instructions in order, so semantics
    are unchanged)."""
    n_split = 0
    for f in nc.m.functions:
        for blk in f.blocks:
            out = []
            changed = False
            for inst in blk.instructions:
                si = inst.sync_info
                waits = list(si.on_wait) if si is not None else []
                if len(waits) > 1:
                    changed = True
                    for j, w in enumerate(waits[:-1]):
                        nop = mybir.InstNoOp(
                            name=f"{inst.name}-wsplit{j}", ins=[], outs=[]
                        )
                        nop.engine = inst.engine
                        nop.sync_info = mybir.SyncInfo(on_wait=[w], on_update=[])
                        out.append(nop)
                        n_split += 1
                    inst.sync_info = mybir.SyncInfo(
                        on_wait=[waits[-1]], on_update=list(si.on_update)
                    )
                out.append(inst)
            if changed:
                blk.instructions = out
    return n_split


def build_kernel(has_bias: bool):
    nc = bass.Bass()
    x = nc.dram_tensor("x", [TOK, D_MODEL], F32, kind="ExternalInput")
    # hi/lo two-level fp8 decomposition of W_proj^T * S_W
    wpt8 = nc.dram_tensor("wpt8", [2, D_MODEL, D_INNER], FP8, kind="ExternalInput")
    wst = nc.dram_tensor("wst", [D_INNER, D_STATE], BF16, kind="ExternalInput")
    wo9 = nc.dram_tensor("wo9", [D_STATE + 1, D_MODEL], F32R, kind="ExternalInput")
    bp = nc.dram_tensor("bp", [1, D_INNER], F32R, kind="ExternalInput")
    b2 = nc.dram_tensor("b2", [D_STATE, 1], F32, kind="ExternalInput")
    ones = nc.dram_tensor("ones", [1, TILE_T], F32R, kind="ExternalInput")
    ident_d = nc.dram_tensor("ident", [P, P], BF16, kind="ExternalInput")
    y = nc.dram_tensor("y", [TOK, D_MODEL], F32, kind="ExternalOutput")

    # all tiles full-size; the last tile overlaps the previous one so the
    # matmuls always stream N=512
    tiles = [(o, TILE_T) for o in range(0, TOK - TILE_T + 1, TILE_T)]
    if tiles[-1][0] + TILE_T < TOK:
        tiles.append((TOK - TILE_T, TILE_T))

    with tile.TileContext(nc) as tc:
        with (
            tc.tile_pool(name="singles", bufs=1) as singles,
            tc.tile_pool(name="xpool", bufs=3) as xpool,
            tc.tile_pool(name="xnpool", bufs=2) as xnpool,
            tc.tile_pool(name="xtpool", bufs=2) as xtpool,
            tc.tile_pool(name="projp", bufs=2) as projp,
            tc.tile_pool(name="outp", bufs=2) as outp,
            tc.tile_pool(name="statp", bufs=6) as statp,
            tc.tile_pool(name="ps_tr", bufs=2, space="PSUM") as ps_tr,
            tc.tile_pool(name="ps1", bufs=3, space="PSUM") as ps1,
            tc.tile_pool(name="ps2", bufs=1, space="PSUM") as ps2,
            tc.tile_pool(name="ps3", bufs=1, space="PSUM") as ps3,
        ):
            wpt_sb = singles.tile([P, 2, KD, D_INNER], FP8)
            wpt_r = wpt8[:, :, :].rearrange("h (k p) e -> p h k e", p=P)
            qs = [nc.sync, nc.scalar]
            for h in range(2):
                for k in range(KD):
                    qs[k % 2].dma_start(wpt_sb[:, h, k], wpt_r[:, h, k])
            wst_sb = singles.tile([P, ME, D_STATE], BF16)
            nc.sync.dma_start(wst_sb, wst[:, :].rearrange("(k p) s -> p k s", p=P))
            wo9_sb = singles.tile([D_STATE + 1, D_MODEL], F32R)
            nc.scalar.dma_start(wo9_sb, wo9[:, :])
            b2_sb = singles.tile([D_STATE, 1], F32)
            nc.sync.dma_start(b2_sb, b2[:, :])
            ident = singles.tile([P, P], BF16)
            nc.scalar.dma_start(ident, ident_d[:, :])
            if has_bias:
                bp_sb = singles.tile([1, D_INNER], F32R)
                nc.sync.dma_start(bp_sb, bp[:, :])
                ones_sb = singles.tile([1, TILE_T], F32R)
                nc.scalar.dma_start(ones_sb, ones[:, :])

            def a_dma(i):
                off, T = tiles[i]
                x_sb = xpool.tile([P, G, D_MODEL], F32, tag="x")
                nc.sync.dma_start(
                    x_sb, x[off : off + T, :].rearrange("(g p) d -> p g d", p=P)
                )
                return x_sb

            def a_ln(x_sb):
                """layernorm one tile -> xn16 (token-major bf16, scaled by
                S_X via rstd = rsqrt((var+eps)/S_X^2)); all on DVE"""
                stats = statp.tile([P, G, 2, 6], F32, tag="bnst")
                mv = statp.tile([P, G, 2], F32, tag="mv")
                for g in range(G):
                    nc.vector.bn_stats(stats[:, g, 0, :], x_sb[:, g, 0:512])
                    nc.vector.bn_stats(stats[:, g, 1, :], x_sb[:, g, 512:1024])
                    nc.vector.bn_aggr(mv[:, g, :], stats[:, g])
                vp = statp.tile([P, G], F32, tag="vp")
                nc.vector.tensor_scalar(
                    out=vp,
                    in0=mv[:, :, 1],
                    scalar1=EPS,
                    scalar2=1.0 / (S_X * S_X),
                    op0=mybir.AluOpType.add,
                    op1=mybir.AluOpType.mult,
                )
                # magic rsqrt + two Newton steps (rel err ~5e-6)
                rs = statp.tile([P, G], F32, tag="rs")
                nc.vector.tensor_scalar(
                    out=rs.bitcast(I32),
                    in0=vp.bitcast(I32),
                    scalar1=1,
                    scalar2=None,
                    op0=mybir.AluOpType.arith_shift_right,
                )
                nc.vector.tensor_scalar(
                    out=rs.bitcast(I32),
                    in0=rs.bitcast(I32),
                    scalar1=-1,
                    scalar2=MAGIC,
                    op0=mybir.AluOpType.mult,
                    op1=mybir.AluOpType.add,
                )
                sq = statp.tile([P, G], F32, tag="sq")
                for _ in range(2):
                    nc.vector.tensor_tensor(
                        out=sq, in0=rs, in1=rs, op=mybir.AluOpType.mult
                    )
                    nc.vector.tensor_tensor(
                        out=sq, in0=sq, in1=vp, op=mybir.AluOpType.mult
                    )
                    nc.vector.tensor_scalar(
                        out=sq,
                        in0=sq,
                        scalar1=-0.5,
                        scalar2=1.5,
                        op0=mybir.AluOpType.mult,
                        op1=mybir.AluOpType.add,
                    )
                    nc.vector.tensor_tensor(
                        out=rs, in0=rs, in1=sq, op=mybir.AluOpType.mult
                    )
                xn_sb = xnpool.tile([P, G, D_MODEL], BF16, tag="xn")
                for g in range(G):
                    nc.vector.tensor_scalar(
                        out=xn_sb[:, g, :],
                        in0=x_sb[:, g, :],
                        scalar1=mv[:, g, 0:1],
                        scalar2=rs[:, g : g + 1],
                        op0=mybir.AluOpType.subtract,
                        op1=mybir.AluOpType.mult,
                    )
                return xn_sb

            def a_tr(xn_sb):
                """PE-transpose to feature-major; ACT copyback casts to
                fp8e4. One PSUM tile and one ACT copy per k-pair."""
                xnT = xtpool.tile([P, KD, G, P], FP8, tag="xnT")
                for r in range(KD // 2):
                    ptr = ps_tr.tile([P, 2, G, P], BF16, tag="ptr")
                    for kk in range(2):
                        k = 2 * r + kk
                        for g in range(G):
                            nc.tensor.transpose(
                                ptr[:, kk, g, :],
                                xn_sb[:, g, k * P : (k + 1) * P],
                                ident,
                            )
                    nc.scalar.activation(
                        out=xnT[:, 2 * r : 2 * r + 2],
                        in_=ptr[:],
                        func=mybir.ActivationFunctionType.Copy,
                    )
                return xnT

            # software pipeline: x-DMA two tiles ahead, LayerNorm one tile
            # ahead, transposes one tile ahead in the M2->M3 pocket
            x_tiles = [a_dma(0), a_dma(1)]
            xn_cur = a_ln(x_tiles[0])
            xnT_cur = a_tr(xn_cur)
            xn_next = a_ln(x_tiles[1])
            for i, (off, T) in enumerate(tiles):
                x_sb = x_tiles[i]
                xnT = xnT_cur
                if i + 2 < len(tiles):
                    x_tiles.append(a_dma(i + 2))
                # cs9 allocated + ones row DMA'd early (row 8 is only
                # reachable by DMA; issuing here hides its latency)
                cs9 = statp.tile([D_STATE + 1, TILE_T], F32R, tag="cs9")
                nc.sync.dma_start(cs9[D_STATE : D_STATE + 1, :], ones[:, :])
                # matmul 1: fp8 DoubleRow over (hi, lo) weight levels;
                # SiLU per m-chunk with the 1/(S_W*S_X) descale fused
                projT = projp.tile([P, ME, TILE_T], BF16, tag="projT")
                for m in range(ME):
                    p1 = ps1.tile([P, TILE_T], F32, tag="p1")
                    if has_bias:
                        nc.tensor.matmul(
                            p1,
                            lhsT=bp_sb[:, m * P : (m + 1) * P],
                            rhs=ones_sb[:, :],
                            start=True,
                            stop=False,
                            skip_group_check=True,
                        )
                    for h in range(2):
                        for r in range(KD // 2):
                            nc.tensor.matmul(
                                p1,
                                lhsT=wpt_sb[
                                    :, h, 2 * r : 2 * r + 2, m * P : (m + 1) * P
                                ],
                                rhs=xnT[:, 2 * r : 2 * r + 2],
                                start=(h == 0 and r == 0 and not has_bias),
                                stop=(h == 1 and r == KD // 2 - 1),
                                perf_mode=mybir.MatmulPerfMode.DoubleRow,
                                skip_group_check=has_bias,
                            )
                    nc.scalar.activation(
                        out=projT[:, m, :],
                        in_=p1,
                        func=mybir.ActivationFunctionType.Silu,
                        bias=0.0,
                        scale=1.0 / (S_W * S_X),
                    )
                # matmul 2: bf16, [D_STATE, T]
                p2 = ps2.tile([D_STATE, TILE_T], F32, tag="p2")
                for k2 in range(ME):
                    nc.tensor.matmul(
                        p2,
                        lhsT=wst_sb[:, k2, :],
                        rhs=projT[:, k2, :],
                        start=(k2 == 0),
                        stop=(k2 == ME - 1),
                    )
                # next tile's transposes fill the PE while ACT drains
                # p2 -> cs9; LN for the tile after runs on DVE
                if i + 1 < len(tiles):
                    xnT_cur = a_tr(xn_next)
                if i + 2 < len(tiles):
                    xn_next = a_ln(x_tiles[i + 2])
                nc.scalar.activation(
                    out=cs9[:D_STATE, :],
                    in_=p2,
                    func=mybir.ActivationFunctionType.Silu,
                    bias=b2_sb,
                    scale=1.0,
                )
                # matmul 3: K=9 (ones row adds b_out), token-major out;
                # residual add fused into the DVE copyback
                out_sb = outp.tile([P, G, D_MODEL], F32, tag="out")
                for g in range(G):
                    p3 = ps3.tile([P, 2, 512], F32, tag="p3")
                    for h in range(D_MODEL // 512):
                        nc.tensor.matmul(
                            p3[:, h, :],
                            lhsT=cs9[:, g * P : (g + 1) * P],
                            rhs=wo9_sb[:, h * 512 : (h + 1) * 512],
                            start=True,
                            stop=True,
                        )
                        nc.vector.tensor_add(
                            out=out_sb[:, g, h * 512 : (h + 1) * 512],
                            in0=p3[:, h, :],
                            in1=x_sb[:, g, h * 512 : (h + 1) * 512],
                        )
                nc.sync.dma_start(
                    y[off : off + T, :].rearrange("(g p) d -> p g d", p=P), out_sb
                )

    _split_multi_waits(nc)
    return nc


_NC_CACHE = {}


def _get_nc(has_bias: bool):
    if has_bias not in _NC_CACHE:
        _NC_CACHE[has_bias] = build_kernel(has_bias)
    return _NC_CACHE[has_bias]


def make_in_maps(inputs):
    x = np.ascontiguousarray(inputs["x"], dtype=np.float32).reshape(-1, D_MODEL)
    W_proj = np.asarray(inputs["W_proj"], dtype=np.float64)
    b_proj = np.asarray(inputs["b_proj"], dtype=np.float64)
    W_state = np.asarray(inputs["W_state"], dtype=np.float32)
    b_state = np.asarray(inputs["b_state"], dtype=np.float32)
    W_out = np.asarray(inputs["W_out"], dtype=np.float32)
    b_out = np.asarray(inputs["b_out"], dtype=np.float32)
    initial_state = np.asarray(inputs["initial_state"], dtype=np.float32)
    gamma = np.asarray(inputs["gamma"], dtype=np.float64)
    beta = np.asarray(inputs["beta"], dtype=np.float64)

    # fold the LayerNorm affine into the projection
    Wp = W_proj * gamma[None, :]  # [d_inner, d_model]
    bp = b_proj + W_proj @ beta  # [d_inner]
    has_bias = bool(np.any(bp != 0.0))

    wpre = np.clip(Wp.T * S_W, -224.0, 224.0)
    w_hi = wpre.astype(ml_dtypes.float8_e4m3)
    w_lo = (wpre - w_hi.astype(np.float64)).astype(ml_dtypes.float8_e4m3)
    wpt8 = np.stack([w_hi, w_lo], axis=0)  # [2, d_model, d_inner]

    shared = {
        "wpt8": np.ascontiguousarray(wpt8),
        "wst": np.ascontiguousarray(W_state.T.astype(ml_dtypes.bfloat16)),
        "wo9": np.ascontiguousarray(
            np.concatenate([W_out.T, b_out[None, :]], axis=0)
        ),
        "bp": np.ascontiguousarray((bp * S_W * S_X).astype(np.float32))[None, :],
        "b2": np.ascontiguousarray(
            (b_state + initial_state.reshape(-1)).reshape(D_STATE, 1)
        ),
        "ones": np.ones((1, TILE_T), dtype=np.float32),
        "ident": np.eye(P, dtype=ml_dtypes.bfloat16),
    }
    in_maps = []
    for c in range(N_CORES):
        m = {"x": np.ascontiguousarray(x[c * TOK : (c + 1) * TOK])}
        m.update(shared)
        in_maps.append(m)
    return in_maps, has_bias


def kernel(**inputs) -> np.ndarray:
    in_maps, has_bias = make_in_maps(inputs)
    nc = _get_nc(has_bias)
    res = run_bass_kernel_spmd(nc, in_maps, core_ids=list(range(N_CORES)))
    out = np.concatenate([res.results[c]["y"] for c in range(N_CORES)], axis=0)
    return out.reshape(np.asarray(inputs["x"]).shape)


# revision 55
# speedup vs baseline: 1.0099x; 1.0023x over previous
"""Trainium2 Bass kernel for the MemoryEfficientMambaBlock problem.

Data-parallel over 8 NeuronCores: x sharded over tokens, small weights
replicated. Per core, per 512-token tile:
  LayerNorm (bn_stats + magic-rsqrt + apply, all on DVE, so the scalar
  engine only ever runs SiLU — no activation-table swaps) -> PE
  transpose to feature-major bf16 -> ACT copyback casting to fp8e4
  (gamma folded into W_proj host-side, beta folded into b_proj) ->
  fp8 DoubleRow matmul x8 @ (W_hi + W_lo) where W_hi/W_lo are a
  two-level e4m3 decomposition of W_proj*64 (kills the weight-side
  quantization noise; 8 pair-MMs per 128-col chunk accumulate in one
  PSUM bank) -> SiLU per m-chunk with the 1/(S_W*S_X) descale fused ->
  bf16 matmul @W_stateT -> SiLU+(b_state+initial_state) -> K=9 f32r
  matmul (ones row carries b_out) with the residual add fused into the
  DVE copyback.
"""

import sys

if "/opt/trn_rl_repo" not in sys.path:
    sys.path.insert(0, "/opt/trn_rl_repo")

import numpy as np
import ml_dtypes

import concourse.bass as bass
import concourse.mybir as mybir
import concourse.tile as tile
from concourse.bass_utils import run_bass_kernel_spmd

P = 128
D_MODEL = 1024
D_INNER = 2048
D_STATE = 8
EPS = 1e-5
N_CORES = 8
TOK_TOTAL = 2 * 128 * 196  # 50176
TOK = TOK_TOTAL // N_CORES  # 6272
TILE_T = 512
G = TILE_T // P  # 4

KD = D_MODEL // P  # 8 contraction chunks for matmul 1
ME = D_INNER // P  # 16 output tiles for matmul 1 / contraction chunks for 2

S_W = 64.0  # fp8 scale on W_proj
S_X = 8.0  # fp8 scale on xn (folded into rstd)

F32 = mybir.dt.float32
F32R = mybir.dt.float32r
BF16 = mybir.dt.bfloat16
FP8 = mybir.dt.float8e4
I32 = mybir.dt.int32

MAGIC = 0x5F3759DF


def _split_multi_waits(nc):
    """This container's walrus accepts at most ONE semaphore wait per
    instruction. Hoist all but the last wait of each instruction onto
    fresh single-wait NoOps inserted immediately before it on the same
    engine (the sequencer processes instructions in order, so semantics
    are unchanged)."""
    n_split = 0
    for f in nc.m.functions:
        for blk in f.blocks:
            out = []
            changed = False
            for inst in blk.instructions:
                si = inst.sync_info
                waits = list(si.on_wait) if si is not None else []
                if len(waits) > 1:
                    changed = True
                    for j, w in enumerate(waits[:-1]):
                        nop = mybir.InstNoOp(
                            name=f"{inst.name}-wsplit{j}", ins=[], outs=[]
                        )
                        nop.engine = inst.engine
                        nop.sync_info = mybir.SyncInfo(on_wait=[w], on_update=[])
                        out.append(nop)
                        n_split += 1
                    inst.sync_info = mybir.SyncInfo(
                        on_wait=[waits[-1]], on_update=list(si.on_update)
                    )
                out.append(inst)
            if changed:
                blk.instructions = out
    return n_split


def build_kernel(has_bias: bool):
    nc = bass.Bass()
    x = nc.dram_tensor("x", [TOK, D_MODEL], F32, kind="ExternalInput")
    # hi/lo two-level fp8 decomposition of W_proj^T * S_W
    wpt8 = nc.dram_tensor("wpt8", [2, D_MODEL, D_INNER], FP8, kind="ExternalInput")
    wst = nc.dram_tensor("wst", [D_INNER, D_STATE], BF16, kind="ExternalInput")
    wo9 = nc.dram_tensor("wo9", [D_STATE + 1, D_MODEL], F32R, kind="ExternalInput")
    bp = nc.dram_tensor("bp", [1, D_INNER], F32R, kind="ExternalInput")
    b2 = nc.dram_tensor("b2", [D_STATE, 1], F32, kind="ExternalInput")
    ones = nc.dram_tensor("ones", [1, TILE_T], F32R, kind="ExternalInput")
    ident_d = nc.dram_tensor("ident", [P, P], BF16, kind="ExternalInput")
    y = nc.dram_tensor("y", [TOK, D_MODEL], F32, kind="ExternalOutput")

    # all tiles full-size; the last tile overlaps the previous one so the
    # matmuls always stream N=512
    tiles = [(o, TILE_T) for o in range(0, TOK - TILE_T + 1, TILE_T)]
    if tiles[-1][0] + TILE_T < TOK:
        tiles.append((TOK - TILE_T, TILE_T))

    with tile.TileContext(nc) as tc:
        with (
            tc.tile_pool(name="singles", bufs=1) as singles,
            tc.tile_pool(name="xpool", bufs=3) as xpool,
            tc.tile_pool(name="xnpool", bufs=2) as xnpool,
            tc.tile_pool(name="xtpool", bufs=2) as xtpool,
            tc.tile_pool(name="projp", bufs=2) as projp,
            tc.tile_pool(name="outp", bufs=2) as outp,
            tc.tile_pool(name="statp", bufs=6) as statp,
            tc.tile_pool(name="ps_tr", bufs=2, space="PSUM") as ps_tr,
            tc.tile_pool(name="ps1", bufs=3, space="PSUM") as ps1,
            tc.tile_pool(name="ps2", bufs=1, space="PSUM") as ps2,
            tc.tile_pool(name="ps3", bufs=1, space="PSUM") as ps3,
        ):
            wpt_sb = singles.tile([P, 2, KD, D_INNER], FP8)
            wpt_r = wpt8[:, :, :].rearrange("h (k p) e -> p h k e", p=P)
            qs = [nc.sync, nc.scalar]
            for h in range(2):
                for k in range(KD):
                    qs[k % 2].dma_start(wpt_sb[:, h, k], wpt_r[:, h, k])
            wst_sb = singles.tile([P, ME, D_STATE], BF16)
            nc.sync.dma_start(wst_sb, wst[:, :].rearrange("(k p) s -> p k s", p=P))
            wo9_sb = singles.tile([D_STATE + 1, D_MODEL], F32R)
            nc.scalar.dma_start(wo9_sb, wo9[:, :])
            b2_sb = singles.tile([D_STATE, 1], F32)
            nc.sync.dma_start(b2_sb, b2[:, :])
            ident = singles.tile([P, P], BF16)
            nc.scalar.dma_start(ident, ident_d[:, :])
            if has_bias:
                bp_sb = singles.tile([1, D_INNER], F32R)
                nc.sync.dma_start(bp_sb, bp[:, :])
                ones_sb = singles.tile([1, TILE_T], F32R)
                nc.scalar.dma_start(ones_sb, ones[:, :])

            def a_dma(i):
                off, T = tiles[i]
                x_sb = xpool.tile([P, G, D_MODEL], F32, tag="x")
                nc.sync.dma_start(
                    x_sb, x[off : off + T, :].rearrange("(g p) d -> p g d", p=P)
                )
                return x_sb

            def a_ln(x_sb):
                """layernorm one tile -> xn16 (token-major bf16, scaled by
                S_X via rstd = rsqrt((var+eps)/S_X^2)); all on DVE"""
                stats = statp.tile([P, G, 2, 6], F32, tag="bnst")
                mv = statp.tile([P, G, 2], F32, tag="mv")
                for g in range(G):
                    nc.vector.bn_stats(stats[:, g, 0, :], x_sb[:, g, 0:512])
                    nc.vector.bn_stats(stats[:, g, 1, :], x_sb[:, g, 512:1024])
                    nc.vector.bn_aggr(mv[:, g, :], stats[:, g])
                vp = statp.tile([P, G], F32, tag="vp")
                nc.vector.tensor_scalar(
                    out=vp,
                    in0=mv[:, :, 1],
                    scalar1=EPS,
                    scalar2=1.0 / (S_X * S_X),
                    op0=mybir.AluOpType.add,
                    op1=mybir.AluOpType.mult,
                )
                # magic rsqrt + two Newton steps (rel err ~5e-6)
                rs = statp.tile([P, G], F32, tag="rs")
                nc.vector.tensor_scalar(
                    out=rs.bitcast(I32),
                    in0=vp.bitcast(I32),
                    scalar1=1,
                    scalar2=None,
                    op0=mybir.AluOpType.arith_shift_right,
                )
                nc.vector.tensor_scalar(
                    out=rs.bitcast(I32),
                    in0=rs.bitcast(I32),
                    scalar1=-1,
                    scalar2=MAGIC,
                    op0=mybir.AluOpType.mult,
                    op1=mybir.AluOpType.add,
                )
                sq = statp.tile([P, G], F32, tag="sq")
                for _ in range(2):
                    nc.vector.tensor_tensor(
                        out=sq, in0=rs, in1=rs, op=mybir.AluOpType.mult
                    )
                    nc.vector.tensor_tensor(
                        out=sq, in0=sq, in1=vp, op=mybir.AluOpType.mult
                    )
                    nc.vector.tensor_scalar(
                        out=sq,
                        in0=sq,
                        scalar1=-0.5,
                        scalar2=1.5,
                        op0=mybir.AluOpType.mult,
                        op1=mybir.AluOpType.add,
                    )
                    nc.vector.tensor_tensor(
                        out=rs, in0=rs, in1=sq, op=mybir.AluOpType.mult
                    )
                xn_sb = xnpool.tile([P, G, D_MODEL], BF16, tag="xn")
                for g in range(G):
                    nc.vector.tensor_scalar(
                        out=xn_sb[:, g, :],
                        in0=x_sb[:, g, :],
                        scalar1=mv[:, g, 0:1],
                        scalar2=rs[:, g : g + 1],
                        op0=mybir.AluOpType.subtract,
                        op1=mybir.AluOpType.mult,
                    )
                return xn_sb

            def a_tr(xn_sb):
                """PE-transpose to feature-major; ACT copyback casts to
                fp8e4. One PSUM tile and one ACT copy per k-pair."""
                xnT = xtpool.tile([P, KD, G, P], FP8, tag="xnT")
                for r in range(KD // 2):
                    ptr = ps_tr.tile([P, 2, G, P], BF16, tag="ptr")
                    for kk in range(2):
                        k = 2 * r + kk
                        for g in range(G):
                            nc.tensor.transpose(
                                ptr[:, kk, g, :],
                                xn_sb[:, g, k * P : (k + 1) * P],
                                ident,
                            )
                    nc.scalar.activation(
                        out=xnT[:, 2 * r : 2 * r + 2],
                        in_=ptr[:],
                        func=mybir.ActivationFunctionType.Copy,
                    )
                return xnT

            # software pipeline: x-DMA two tiles ahead, LayerNorm one tile
            # ahead, transposes one tile ahead in the M2->M3 pocket
            x_tiles = [a_dma(0), a_dma(1)]
            xn_cur = a_ln(x_tiles[0])
            xnT_cur = a_tr(xn_cur)
            xn_next = a_ln(x_tiles[1])
            for i, (off, T) in enumerate(tiles):
                x_sb = x_tiles[i]
                xnT = xnT_cur
                if i + 2 < len(tiles):
                    x_tiles.append(a_dma(i + 2))
                # cs9 allocated + ones row DMA'd early (row 8 is only
                # reachable by DMA; issuing here hides its latency)
                cs9 = statp.tile([D_STATE + 1, TILE_T], F32R, tag="cs9")
                nc.sync.dma_start(cs9[D_STATE : D_STATE + 1, :], ones[:, :])
                # matmul 1: fp8 DoubleRow over (hi, lo) weight levels;
                # SiLU per m-chunk with the 1/(S_W*S_X) descale fused
                projT = projp.tile([P, ME, TILE_T], BF16, tag="projT")
                for m in range(ME):
                    p1 = ps1.tile([P, TILE_T], F32, tag="p1")
                    if has_bias:
                        nc.tensor.matmul(
                            p1,
                            lhsT=bp_sb[:, m * P : (m + 1) * P],
                            rhs=ones_sb[:, :],
                            start=True,
                            stop=False,
                            skip_group_check=True,
                        )
                    for h in range(2):
                        for r in range(KD // 2):
                            nc.tensor.matmul(
                                p1,
                                lhsT=wpt_sb[
                                    :, h, 2 * r : 2 * r + 2, m * P : (m + 1) * P
                                ],
                                rhs=xnT[:, 2 * r : 2 * r + 2],
                                start=(h == 0 and r == 0 and not has_bias),
                                stop=(h == 1 and r == KD // 2 - 1),
                                perf_mode=mybir.MatmulPerfMode.DoubleRow,
                                skip_group_check=has_bias,
                            )
                    nc.scalar.activation(
                        out=projT[:, m, :],
                        in_=p1,
                        func=mybir.ActivationFunctionType.Silu,
                        bias=0.0,
                        scale=1.0 / (S_W * S_X),
                    )
                # matmul 2: bf16, [D_STATE, T]
                p2 = ps2.tile([D_STATE, TILE_T], F32, tag="p2")
                for k2 in range(ME):
                    nc.tensor.matmul(
                        p2,
                        lhsT=wst_sb[:, k2, :],
                        rhs=projT[:, k2, :],
                        start=(k2 == 0),
                        stop=(k2 == ME - 1),
                    )
                # next tile's transposes fill the PE while ACT drains
                # p2 -> cs9
                if i + 1 < len(tiles):
                    xnT_cur = a_tr(xn_next)
                nc.scalar.activation(
                    out=cs9[:D_STATE, :],
                    in_=p2,
                    func=mybir.ActivationFunctionType.Silu,
                    bias=b2_sb,
                    scale=1.0,
                )
                # matmul 3: K=9 (ones row adds b_out), token-major out;
                # residual add fused into the DVE copyback
                out_sb = outp.tile([P, G, D_MODEL], F32, tag="out")
                for g in range(G):
                    p3 = ps3.tile([P, 2, 512], F32, tag="p3")
                    for h in range(D_MODEL // 512):
                        nc.tensor.matmul(
                            p3[:, h, :],
                            lhsT=cs9[:, g * P : (g + 1) * P],
                            rhs=wo9_sb[:, h * 512 : (h + 1) * 512],
                            start=True,
                            stop=True,
                        )
                        nc.vector.tensor_add(
                            out=out_sb[:, g, h * 512 : (h + 1) * 512],
                            in0=p3[:, h, :],
                            in1=x_sb[:, g, h * 512 : (h + 1) * 512],
                        )
                nc.sync.dma_start(
                    y[off : off + T, :].rearrange("(g p) d -> p g d", p=P), out_sb
                )
                # LN for the tile after is emitted BEHIND this tile's
                # residual adds: the DVE queue is in-order, and parking
                # ~10us of LN work ahead of the resid TTs would stall
                # matmul 3 on the ps3 rotation
                if i + 2 < len(tiles):
                    xn_next = a_ln(x_tiles[i + 2])

    _split_multi_waits(nc)
    return nc


_NC_CACHE = {}


def _get_nc(has_bias: bool):
    if has_bias not in _NC_CACHE:
        _NC_CACHE[has_bias] = build_kernel(has_bias)
    return _NC_CACHE[has_bias]


def make_in_maps(inputs):
    x = np.ascontiguousarray(inputs["x"], dtype=np.float32).reshape(-1, D_MODEL)
    W_proj = np.asarray(inputs["W_proj"], dtype=np.float64)
    b_proj = np.asarray(inputs["b_proj"], dtype=np.float64)
    W_state = np.asarray(inputs["W_state"], dtype=np.float32)
    b_state = np.asarray(inputs["b_state"], dtype=np.float32)
    W_out = np.asarray(inputs["W_out"], dtype=np.float32)
    b_out = np.asarray(inputs["b_out"], dtype=np.float32)
    initial_state = np.asarray(inputs["initial_state"], dtype=np.float32)
    gamma = np.asarray(inputs["gamma"], dtype=np.float64)
    beta = np.asarray(inputs["beta"], dtype=np.float64)

    # fold the LayerNorm affine into the projection
    Wp = W_proj * gamma[None, :]  # [d_inner, d_model]
    bp = b_proj + W_proj @ beta  # [d_inner]
    has_bias = bool(np.any(bp != 0.0))

    wpre = np.clip(Wp.T * S_W, -224.0, 224.0)
    w_hi = wpre.astype(ml_dtypes.float8_e4m3)
    w_lo = (wpre - w_hi.astype(np.float64)).astype(ml_dtypes.float8_e4m3)
    wpt8 = np.stack([w_hi, w_lo], axis=0)  # [2, d_model, d_inner]

    shared = {
        "wpt8": np.ascontiguousarray(wpt8),
        "wst": np.ascontiguousarray(W_state.T.astype(ml_dtypes.bfloat16)),
        "wo9": np.ascontiguousarray(
            np.concatenate([W_out.T, b_out[None, :]], axis=0)
        ),
        "bp": np.ascontiguousarray((bp * S_W * S_X).astype(np.float32))[None, :],
        "b2": np.ascontiguousarray(
            (b_state + initial_state.reshape(-1)).reshape(D_STATE, 1)
        ),
        "ones": np.ones((1, TILE_T), dtype=np.float32),
        "ident": np.eye(P, dtype=ml_dtypes.bfloat16),
    }
    in_maps = []
    for c in range(N_CORES):
        m = {"x": np.ascontiguousarray(x[c * TOK : (c + 1) * TOK])}
        m.update(shared)
        in_maps.append(m)
    return in_maps, has_bias


def kernel(**inputs) -> np.ndarray:
    in_maps, has_bias = make_in_maps(inputs)
    nc = _get_nc(has_bias)
    res = run_bass_kernel_spmd(nc, in_maps, core_ids=list(range(N_CORES)))
    out = np.concatenate([res.results[c]["y"] for c in range(N_CORES)], axis=0)
    return out.reshape(np.asarray(inputs["x"]).shape)


# revision 57
# speedup vs baseline: 1.0721x; 1.0615x over previous
"""Trainium2 Bass kernel for the MemoryEfficientMambaBlock problem.

Data-parallel over 8 NeuronCores: x sharded over tokens, small weights
replicated. Per core, per 512-token tile:
  LayerNorm (bn_stats + magic-rsqrt + apply, all on DVE, so the scalar
  engine only ever runs SiLU — no activation-table swaps) -> PE
  transpose to feature-major bf16 -> ACT copyback casting to fp8e4
  (gamma folded into W_proj host-side, beta folded into b_proj) ->
  fp8 DoubleRow matmul x8 @ (W_hi + W_lo) where W_hi/W_lo are a
  two-level e4m3 decomposition of W_proj*64 (kills the weight-side
  quantization noise; 8 pair-MMs per 128-col chunk accumulate in one
  PSUM bank) -> SiLU per m-chunk with the 1/(S_W*S_X) descale fused ->
  bf16 matmul @W_stateT -> SiLU+(b_state+initial_state) -> K=9 f32r
  matmul (ones row carries b_out) with the residual add fused into the
  DVE copyback.
"""

import sys

if "/opt/trn_rl_repo" not in sys.path:
    sys.path.insert(0, "/opt/trn_rl_repo")

import numpy as np
import ml_dtypes

import concourse.bass as bass
import concourse.mybir as mybir
import concourse.tile as tile
from concourse.bass_utils import run_bass_kernel_spmd

P = 128
D_MODEL = 1024
D_INNER = 2048
D_STATE = 8
EPS = 1e-5
N_CORES = 8
TOK_TOTAL = 2 * 128 * 196  # 50176
TOK = TOK_TOTAL // N_CORES  # 6272
TILE_T = 512
G = TILE_T // P  # 4

KD = D_MODEL // P  # 8 contraction chunks for matmul 1
ME = D_INNER // P  # 16 output tiles for matmul 1 / contraction chunks for 2

S_W = 64.0  # fp8 scale on W_proj
S_X = 8.0  # fp8 scale on xn (folded into rstd)

F32 = mybir.dt.float32
F32R = mybir.dt.float32r
BF16 = mybir.dt.bfloat16
FP8 = mybir.dt.float8e4
I32 = mybir.dt.int32

MAGIC = 0x5F3759DF


def _split_multi_waits(nc):
    """This container's walrus accepts at most ONE semaphore wait per
    instruction. Hoist all but the last wait of each instruction onto
    fresh single-wait NoOps inserted immediately before it on the same
    engine (the sequencer processes instructions in order, so semantics
    are unchanged)."""
    n_split = 0
    for f in nc.m.functions:
        for blk in f.blocks:
            out = []
            changed = False
            for inst in blk.instructions:
                si = inst.sync_info
                waits = list(si.on_wait) if si is not None else []
                if len(waits) > 1:
                    changed = True
                    for j, w in enumerate(waits[:-1]):
                        nop = mybir.InstNoOp(
                            name=f"{inst.name}-wsplit{j}", ins=[], outs=[]
                        )
                        nop.engine = inst.engine
                        nop.sync_info = mybir.SyncInfo(on_wait=[w], on_update=[])
                        out.append(nop)
                        n_split += 1
                    inst.sync_info = mybir.SyncInfo(
                        on_wait=[waits[-1]], on_update=list(si.on_update)
                    )
                out.append(inst)
            if changed:
                blk.instructions = out
    return n_split


def build_kernel(has_bias: bool):
    nc = bass.Bass()
    x = nc.dram_tensor("x", [TOK, D_MODEL], F32, kind="ExternalInput")
    # hi/lo two-level fp8 decomposition of W_proj^T * S_W
    wpt8 = nc.dram_tensor("wpt8", [2, D_MODEL, D_INNER], FP8, kind="ExternalInput")
    wst = nc.dram_tensor("wst", [D_INNER, D_STATE], BF16, kind="ExternalInput")
    wo9 = nc.dram_tensor("wo9", [D_STATE + 1, D_MODEL], F32R, kind="ExternalInput")
    bp = nc.dram_tensor("bp", [1, D_INNER], F32R, kind="ExternalInput")
    b2 = nc.dram_tensor("b2", [D_STATE, 1], F32, kind="ExternalInput")
    ones = nc.dram_tensor("ones", [1, TILE_T], F32R, kind="ExternalInput")
    ident_d = nc.dram_tensor("ident", [P, P], BF16, kind="ExternalInput")
    y = nc.dram_tensor("y", [TOK, D_MODEL], F32, kind="ExternalOutput")

    # all tiles full-size; the last tile overlaps the previous one so the
    # matmuls always stream N=512
    tiles = [(o, TILE_T) for o in range(0, TOK - TILE_T + 1, TILE_T)]
    if tiles[-1][0] + TILE_T < TOK:
        tiles.append((TOK - TILE_T, TILE_T))

    with tile.TileContext(nc) as tc:
        with (
            tc.tile_pool(name="singles", bufs=1) as singles,
            tc.tile_pool(name="xpool", bufs=3) as xpool,
            tc.tile_pool(name="xnpool", bufs=2) as xnpool,
            tc.tile_pool(name="xtpool", bufs=2) as xtpool,
            tc.tile_pool(name="projp", bufs=2) as projp,
            tc.tile_pool(name="outp", bufs=2) as outp,
            tc.tile_pool(name="statp", bufs=6) as statp,
            tc.tile_pool(name="ps_tr", bufs=2, space="PSUM") as ps_tr,
            tc.tile_pool(name="ps1", bufs=3, space="PSUM") as ps1,
            tc.tile_pool(name="ps2", bufs=1, space="PSUM") as ps2,
            tc.tile_pool(name="ps3", bufs=1, space="PSUM") as ps3,
        ):
            wpt_sb = singles.tile([P, 2, KD, D_INNER], FP8)
            wpt_r = wpt8[:, :, :].rearrange("h (k p) e -> p h k e", p=P)
            qs = [nc.sync, nc.scalar]
            for h in range(2):
                for k in range(KD):
                    qs[k % 2].dma_start(wpt_sb[:, h, k], wpt_r[:, h, k])
            wst_sb = singles.tile([P, ME, D_STATE], BF16)
            nc.sync.dma_start(wst_sb, wst[:, :].rearrange("(k p) s -> p k s", p=P))
            wo9_sb = singles.tile([D_STATE + 1, D_MODEL], F32R)
            nc.scalar.dma_start(wo9_sb, wo9[:, :])
            b2_sb = singles.tile([D_STATE, 1], F32)
            nc.sync.dma_start(b2_sb, b2[:, :])
            ident = singles.tile([P, P], BF16)
            nc.scalar.dma_start(ident, ident_d[:, :])
            if has_bias:
                bp_sb = singles.tile([1, D_INNER], F32R)
                nc.sync.dma_start(bp_sb, bp[:, :])
                ones_sb = singles.tile([1, TILE_T], F32R)
                nc.scalar.dma_start(ones_sb, ones[:, :])

            def a_dma(i):
                off, T = tiles[i]
                x_sb = xpool.tile([P, G, D_MODEL], F32, tag="x")
                nc.sync.dma_start(
                    x_sb, x[off : off + T, :].rearrange("(g p) d -> p g d", p=P)
                )
                return x_sb

            def a_ln(x_sb):
                """layernorm one tile -> xn16 (token-major bf16, scaled by
                S_X via rstd = rsqrt((var+eps)/S_X^2)); all on DVE"""
                stats = statp.tile([P, G, 2, 6], F32, tag="bnst")
                mv = statp.tile([P, G, 2], F32, tag="mv")
                for g in range(G):
                    nc.vector.bn_stats(stats[:, g, 0, :], x_sb[:, g, 0:512])
                    nc.vector.bn_stats(stats[:, g, 1, :], x_sb[:, g, 512:1024])
                    nc.vector.bn_aggr(mv[:, g, :], stats[:, g])
                vp = statp.tile([P, G], F32, tag="vp")
                nc.vector.tensor_scalar(
                    out=vp,
                    in0=mv[:, :, 1],
                    scalar1=EPS,
                    scalar2=1.0 / (S_X * S_X),
                    op0=mybir.AluOpType.add,
                    op1=mybir.AluOpType.mult,
                )
                # magic rsqrt + two Newton steps (rel err ~5e-6)
                rs = statp.tile([P, G], F32, tag="rs")
                nc.vector.tensor_scalar(
                    out=rs.bitcast(I32),
                    in0=vp.bitcast(I32),
                    scalar1=1,
                    scalar2=None,
                    op0=mybir.AluOpType.arith_shift_right,
                )
                nc.vector.tensor_scalar(
                    out=rs.bitcast(I32),
                    in0=rs.bitcast(I32),
                    scalar1=-1,
                    scalar2=MAGIC,
                    op0=mybir.AluOpType.mult,
                    op1=mybir.AluOpType.add,
                )
                sq = statp.tile([P, G], F32, tag="sq")
                for _ in range(2):
                    nc.vector.tensor_tensor(
                        out=sq, in0=rs, in1=rs, op=mybir.AluOpType.mult
                    )
                    nc.vector.tensor_tensor(
                        out=sq, in0=sq, in1=vp, op=mybir.AluOpType.mult
                    )
                    nc.vector.tensor_scalar(
                        out=sq,
                        in0=sq,
                        scalar1=-0.5,
                        scalar2=1.5,
                        op0=mybir.AluOpType.mult,
                        op1=mybir.AluOpType.add,
                    )
                    nc.vector.tensor_tensor(
                        out=rs, in0=rs, in1=sq, op=mybir.AluOpType.mult
                    )
                xn_sb = xnpool.tile([P, G, D_MODEL], BF16, tag="xn")
                for g in range(G):
                    nc.vector.tensor_scalar(
                        out=xn_sb[:, g, :],
                        in0=x_sb[:, g, :],
                        scalar1=mv[:, g, 0:1],
                        scalar2=rs[:, g : g + 1],
                        op0=mybir.AluOpType.subtract,
                        op1=mybir.AluOpType.mult,
                    )
                return xn_sb

            def a_tr(xn_sb):
                """PE-transpose to feature-major; ACT copyback casts to
                fp8e4. One PSUM tile and one ACT copy per k-pair."""
                xnT = xtpool.tile([P, KD, G, P], FP8, tag="xnT")
                for r in range(KD // 2):
                    ptr = ps_tr.tile([P, 2, G, P], BF16, tag="ptr")
                    for kk in range(2):
                        k = 2 * r + kk
                        for g in range(G):
                            nc.tensor.transpose(
                                ptr[:, kk, g, :],
                                xn_sb[:, g, k * P : (k + 1) * P],
                                ident,
                            )
                    nc.scalar.activation(
                        out=xnT[:, 2 * r : 2 * r + 2],
                        in_=ptr[:],
                        func=mybir.ActivationFunctionType.Copy,
                    )
                return xnT

            # software pipeline: x-DMA two tiles ahead, LayerNorm one tile
            # ahead, transposes one tile ahead in the M2->M3 pocket
            x_tiles = [a_dma(0), a_dma(1)]
            xn_cur = a_ln(x_tiles[0])
            xnT_cur = a_tr(xn_cur)
            xn_next = a_ln(x_tiles[1])
            for i, (off, T) in enumerate(tiles):
                x_sb = x_tiles[i]
                xnT = xnT_cur
                if i + 2 < len(tiles):
                    x_tiles.append(a_dma(i + 2))
                # the final overlapped tile recomputes only its genuinely
                # new tokens (the trailing g-groups) through the matmuls;
                # LN/transpose of the overlap region is off critical path
                is_ov = i == len(tiles) - 1 and TOK % TILE_T != 0
                g_lo = (TILE_T - TOK % TILE_T) // P if is_ov else 0
                nT = T - g_lo * P
                # cs9 allocated + ones row DMA'd early (row 8 is only
                # reachable by DMA; issuing here hides its latency)
                cs9 = statp.tile([D_STATE + 1, TILE_T], F32R, tag="cs9")
                nc.sync.dma_start(
                    cs9[D_STATE : D_STATE + 1, :nT], ones[:, :nT]
                )
                # matmul 1: fp8 DoubleRow over (hi, lo) weight levels;
                # SiLU per m-chunk with the 1/(S_W*S_X) descale fused
                projT = projp.tile([P, ME, TILE_T], BF16, tag="projT")
                for m in range(ME):
                    p1 = ps1.tile([P, TILE_T], F32, tag="p1")
                    if has_bias:
                        nc.tensor.matmul(
                            p1[:, :nT],
                            lhsT=bp_sb[:, m * P : (m + 1) * P],
                            rhs=ones_sb[:, :nT],
                            start=True,
                            stop=False,
                            skip_group_check=True,
                        )
                    for h in range(2):
                        for r in range(KD // 2):
                            nc.tensor.matmul(
                                p1[:, :nT],
                                lhsT=wpt_sb[
                                    :, h, 2 * r : 2 * r + 2, m * P : (m + 1) * P
                                ],
                                rhs=xnT[:, 2 * r : 2 * r + 2, g_lo:, :],
                                start=(h == 0 and r == 0 and not has_bias),
                                stop=(h == 1 and r == KD // 2 - 1),
                                perf_mode=mybir.MatmulPerfMode.DoubleRow,
                                skip_group_check=has_bias,
                            )
                    nc.scalar.activation(
                        out=projT[:, m, :nT],
                        in_=p1[:, :nT],
                        func=mybir.ActivationFunctionType.Silu,
                        bias=0.0,
                        scale=1.0 / (S_W * S_X),
                    )
                # matmul 2: bf16, [D_STATE, T]
                p2 = ps2.tile([D_STATE, TILE_T], F32, tag="p2")
                for k2 in range(ME):
                    nc.tensor.matmul(
                        p2[:, :nT],
                        lhsT=wst_sb[:, k2, :],
                        rhs=projT[:, k2, :nT],
                        start=(k2 == 0),
                        stop=(k2 == ME - 1),
                    )
                # next tile's transposes fill the PE while ACT drains
                # p2 -> cs9
                if i + 1 < len(tiles):
                    xnT_cur = a_tr(xn_next)
                nc.scalar.activation(
                    out=cs9[:D_STATE, :nT],
                    in_=p2[:, :nT],
                    func=mybir.ActivationFunctionType.Silu,
                    bias=b2_sb,
                    scale=1.0,
                )
                # matmul 3: K=9 (ones row adds b_out), token-major out;
                # residual add fused into the DVE copyback
                out_sb = outp.tile([P, G, D_MODEL], F32, tag="out")
                for g in range(g_lo, G):
                    p3 = ps3.tile([P, 2, 512], F32, tag="p3")
                    for h in range(D_MODEL // 512):
                        nc.tensor.matmul(
                            p3[:, h, :],
                            lhsT=cs9[:, (g - g_lo) * P : (g - g_lo + 1) * P],
                            rhs=wo9_sb[:, h * 512 : (h + 1) * 512],
                            start=True,
                            stop=True,
                        )
                        nc.vector.tensor_add(
                            out=out_sb[:, g, h * 512 : (h + 1) * 512],
                            in0=p3[:, h, :],
                            in1=x_sb[:, g, h * 512 : (h + 1) * 512],
                        )
                nc.sync.dma_start(
                    y[off + g_lo * P : off + T, :].rearrange(
                        "(g p) d -> p g d", p=P
                    ),
                    out_sb[:, g_lo:, :],
                )
                # LN for the tile after is emitted BEHIND this tile's
                # residual adds: the DVE queue is in-order, and parking
                # ~10us of LN work ahead of the resid TTs would stall
                # matmul 3 on the ps3 rotation
                if i + 2 < len(tiles):
                    xn_next = a_ln(x_tiles[i + 2])

    _split_multi_waits(nc)
    return nc


_NC_CACHE = {}


def _get_nc(has_bias: bool):
    if has_bias not in _NC_CACHE:
        _NC_CACHE[has_bias] = build_kernel(has_bias)
    return _NC_CACHE[has_bias]


def make_in_maps(inputs):
    x = np.ascontiguousarray(inputs["x"], dtype=np.float32).reshape(-1, D_MODEL)
    W_proj = np.asarray(inputs["W_proj"], dtype=np.float64)
    b_proj = np.asarray(inputs["b_proj"], dtype=np.float64)
    W_state = np.asarray(inputs["W_state"], dtype=np.float32)
    b_state = np.asarray(inputs["b_state"], dtype=np.float32)
    W_out = np.asarray(inputs["W_out"], dtype=np.float32)
    b_out = np.asarray(inputs["b_out"], dtype=np.float32)
    initial_state = np.asarray(inputs["initial_state"], dtype=np.float32)
    gamma = np.asarray(inputs["gamma"], dtype=np.float64)
    beta = np.asarray(inputs["beta"], dtype=np.float64)

    # fold the LayerNorm affine into the projection
    Wp = W_proj * gamma[None, :]  # [d_inner, d_model]
    bp = b_proj + W_proj @ beta  # [d_inner]
    has_bias = bool(np.any(bp != 0.0))

    wpre = np.clip(Wp.T * S_W, -224.0, 224.0)
    w_hi = wpre.astype(ml_dtypes.float8_e4m3)
    w_lo = (wpre - w_hi.astype(np.float64)).astype(ml_dtypes.float8_e4m3)
    wpt8 = np.stack([w_hi, w_lo], axis=0)  # [2, d_model, d_inner]

    shared = {
        "wpt8": np.ascontiguousarray(wpt8),
        "wst": np.ascontiguousarray(W_state.T.astype(ml_dtypes.bfloat16)),
        "wo9": np.ascontiguousarray(
            np.concatenate([W_out.T, b_out[None, :]], axis=0)
        ),
        "bp": np.ascontiguousarray((bp * S_W * S_X).astype(np.float32))[None, :],
        "b2": np.ascontiguousarray(
            (b_state + initial_state.reshape(-1)).reshape(D_STATE, 1)
        ),
        "ones": np.ones((1, TILE_T), dtype=np.float32),
        "ident": np.eye(P, dtype=ml_dtypes.bfloat16),
    }
    in_maps = []
    for c in range(N_CORES):
        m = {"x": np.ascontiguousarray(x[c * TOK : (c + 1) * TOK])}
        m.update(shared)
        in_maps.append(m)
    return in_maps, has_bias


def kernel(**inputs) -> np.ndarray:
    in_maps, has_bias = make_in_maps(inputs)
    nc = _get_nc(has_bias)
    res = run_bass_kernel_spmd(nc, in_maps, core_ids=list(range(N_CORES)))
    out = np.concatenate([res.results[c]["y"] for c in range(N_CORES)], axis=0)
    return out.reshape(np.asarray(inputs["x"]).shape)
